# revision 12
# baseline (speedup 1.0000x reference)
"""Trainium2 Bass kernel for AtomTransformerBlock (sliding-window attention, W=64).

Sharding: 8 cores x 256 contiguous query atoms (sequence parallel), each core
gets a 64-atom halo of the singles and the corresponding pair-bias band strip.
No collectives needed (halo recompute). All heavy compute in bf16.

v2: packed weight/vec/singles DMAs, full pair prefetch, mean-subtraction folded
into the pair projection weights on host (bias = rs*(pair@wp') + c0), batched
bias-transpose scaling, winadd folded into the bias tile.
"""

import sys

sys.path.insert(0, "/opt/trn_rl_repo")

import numpy as np
import ml_dtypes

import concourse.bass as bass
import concourse.tile as tile
from concourse import bacc, masks, mybir
from concourse.bass_utils import run_bass_kernel_spmd

BF16 = ml_dtypes.bfloat16
F32 = np.float32

N = 2048
C = 128
CP = 16
H = 8
D = 16
WH = 64
NCORES = 8
NQ = N // NCORES          # 256 queries per core
NW = NQ + 2 * WH          # 384 window positions per core
NWC = NW // 128           # 3 w-chunks
NQG = NQ // 16            # 16 query groups of 16
EPS = 1e-5
NEG = -30000.0

dt = mybir.dt
AF = mybir.ActivationFunctionType
ALU = mybir.AluOpType
AX = mybir.AxisListType

WNAMES = [
    "w_gate", "w_skip", "wq_a", "wq_b", "wk_a", "wk_b", "wv",
    "wg_a", "wg_b", "wo_a", "wo_b", "w_og",
    "w_tgate", "w_tskip", "w_sw0", "w_sw1", "w_hd0", "w_hd1",
    "w_to0", "w_to1", "w_tog", "wblk_a", "wblk_b",
]
VNAMES = ["gate_b", "bg_a", "bg_b", "bo", "og_b", "t_gate_b", "tog_b"]


def build_graph(consts, skip=()):
    nc = bacc.Bacc(
        "TRN2",
        target_bir_lowering=False,
        debug=False,
        enable_asserts=False,
        num_devices=NCORES,
    )

    def inp(name, shape, dtype):
        return nc.dram_tensor(name, shape, dtype, kind="ExternalInput")

    # Per-core data (packed)
    scat_d = inp("scat", [128, 6 * 128], dt.float32)       # (sp,asr) x 3 row-tiles
    pair_w = inp("pair_w", [NWC, 128, NQ * CP], dt.bfloat16)
    pk_a = inp("pk_a", [NWC, 128, NQG * 128], dt.bfloat16)
    pk_b = inp("pk_b", [NWC, 128, NQG * 128], dt.bfloat16)
    winadd = inp("winadd", [128, NWC * NQ], dt.bfloat16)
    wcat_d = inp("wcat", [128, len(WNAMES) * 128], dt.bfloat16)
    vcat_d = inp("vcat", [128, len(VNAMES)], dt.float32)

    out_ext = nc.dram_tensor("out", [NQ, C], dt.float32, kind="ExternalOutput")

    c0 = consts["c0"]  # [H] python floats

    with tile.TileContext(nc) as tc:
        with (
            nc.allow_low_precision(reason="bf16 compute; tolerance 2e-2"),
            tc.tile_pool(name="const", bufs=1) as constp,
            tc.tile_pool(name="wpool", bufs=1) as wpool,
            tc.tile_pool(name="sing", bufs=1) as sing,
            tc.tile_pool(name="sbig", bufs=1) as sbig,
            tc.tile_pool(name="stmp", bufs=3) as stmp,
            tc.tile_pool(name="pairp", bufs=3) as pairp,
            tc.tile_pool(name="sqp", bufs=3) as sqp,
            tc.tile_pool(name="zsb", bufs=3) as zsb,
            tc.tile_pool(name="trgn", bufs=2, space="PSUM") as trgn,
            tc.tile_pool(name="spsum", bufs=2, space="PSUM") as spsum,
            tc.tile_pool(name="sbp", bufs=2, space="PSUM") as sbp,
        ):
            # --- packed input loads (few, large DMAs) ---
            scat = wpool.tile([128, 6 * 128], dt.float32, tag="scat")
            nc.sync.dma_start(out=scat[:], in_=scat_d[:])
            wcat = wpool.tile([128, len(WNAMES) * 128], dt.bfloat16, tag="wcat")
            nc.sync.dma_start(out=wcat[:], in_=wcat_d[:])
            vcat = wpool.tile([128, len(VNAMES)], dt.float32, tag="vcat")
            nc.sync.dma_start(out=vcat[:], in_=vcat_d[:])
            wa_cat = wpool.tile([128, NWC * NQ], dt.bfloat16, tag="wa_cat")
            nc.sync.dma_start(out=wa_cat[:], in_=winadd[:])

            # prefetch ALL pair data immediately (dedicated bufs => no gating)
            pw_tiles, pka_tiles, pkb_tiles = [], [], []
            for wc in range(NWC):
                pka = pairp.tile([128, NQG * 128], dt.bfloat16, tag="pka")
                nc.sync.dma_start(out=pka[:], in_=pk_a[wc])
                pka_tiles.append(pka)
                pkb = pairp.tile([128, NQG * 128], dt.bfloat16, tag="pkb")
                nc.sync.dma_start(out=pkb[:], in_=pk_b[wc])
                pkb_tiles.append(pkb)
            for wc in range(NWC):
                pwt = pairp.tile([128, NQ * CP], dt.bfloat16, tag="pair_w")
                nc.sync.dma_start(out=pwt[:], in_=pair_w[wc])
                pw_tiles.append(pwt)

            # --- constants ---
            zero_c = constp.tile([128, 1], dt.float32, tag="zero_c")
            nc.vector.memset(zero_c[:], 0.0)
            eps_c = constp.tile([128, 1], dt.float32, tag="eps_c")
            nc.vector.memset(eps_c[:], EPS)
            nc.const_aps.aps[(dt.float32, 0.0)] = zero_c[:]
            nc.const_aps.aps[(dt.float32, EPS)] = eps_c[:]
            ident = constp.tile([128, 128], dt.bfloat16)
            masks.make_identity(nc, ident[:])
            ident_f = constp.tile([128, 128], dt.float32, tag="ident_f")
            masks.make_identity(nc, ident_f[:])

            wsb = {k: wcat[:, i * 128 : (i + 1) * 128] for i, k in enumerate(WNAMES)}
            vsb = {k: vcat[:, i : i + 1] for i, k in enumerate(VNAMES)}
            c0_sb = wpool.tile([128, H], dt.float32, tag="c0")
            for h in range(H):
                nc.gpsimd.memset(c0_sb[:, h : h + 1], float(c0[h]))

            # ============ singles path ============
            # rows-on-partition LN, then PE-transpose into [C, rows] chain layout
            sn_T = sing.tile([128, NW], dt.bfloat16, tag="sn_T")
            lna_T = sing.tile([128, NW], dt.bfloat16, tag="lna_T")
            sp_T = sing.tile([128, NW], dt.bfloat16, tag="sp_T")
            for rt in range(NWC):
                rows = slice(rt * 128, (rt + 1) * 128)
                for ti, (nm, dstT, do_raw) in enumerate(
                    (("sp", sn_T, True), ("asr", lna_T, False))
                ):
                    xt = scat[:, (rt * 2 + ti) * 128 : (rt * 2 + ti + 1) * 128]
                    stats = stmp.tile([128, 6], dt.float32, tag="ln_stats")
                    nc.vector.bn_stats(out=stats[:], in_=xt)
                    mv = stmp.tile([128, 2], dt.float32, tag="ln_mv")
                    nc.vector.bn_aggr(out=mv[:], in_=stats[:])
                    sd = stmp.tile([128, 1], dt.float32, tag="ln_sd")
                    nc.scalar.activation(sd[:], mv[:, 1:2], AF.Sqrt, bias=EPS)
                    rsv = stmp.tile([128, 1], dt.float32, tag="ln_rs")
                    nc.vector.reciprocal_approx_fast(rsv[:], sd[:])
                    nmrs = stmp.tile([128, 1], dt.float32, tag="ln_nm")
                    nc.vector.scalar_tensor_tensor(
                        nmrs[:], mv[:, 0:1], -1.0, rsv[:], ALU.mult, ALU.mult
                    )
                    lnx = stmp.tile([128, 128], dt.bfloat16, tag="ln_out")
                    nc.scalar.activation(
                        lnx[:], xt, AF.Identity, bias=nmrs[:], scale=rsv[:]
                    )
                    tp = trgn.tile([128, 128], dt.bfloat16, tag="rgn")
                    nc.tensor.transpose(tp[:], lnx[:], ident[:])
                    nc.scalar.copy(dstT[:, rows], tp[:])
                    if do_raw:
                        tpr = trgn.tile([128, 128], dt.bfloat16, tag="rgn")
                        xb = stmp.tile([128, 128], dt.bfloat16, tag="ln_xb")
                        nc.vector.tensor_copy(xb[:], xt)
                        nc.tensor.transpose(tpr[:], xb[:], ident[:])
                        nc.scalar.copy(sp_T[:, rows], tpr[:])

            # squares of pair (scalar engine for wc0/2, gpsimd wc1: keep DVE free)
            sq_tiles = []
            for wc in range(NWC):
                sq = sqp.tile([128, NQ * CP], dt.bfloat16, tag="sq")
                if wc == 1:
                    nc.gpsimd.tensor_tensor(
                        sq[:], pw_tiles[wc][:], pw_tiles[wc][:], op=ALU.mult
                    )
                else:
                    nc.scalar.square(sq[:], pw_tiles[wc][:])
                sq_tiles.append(sq)
            qsl = slice(WH, WH + NQ)  # center 256 of the 384 halo rows

            def mm(wname, rhs_ap, n, tag):
                ps = spsum.tile([128, n], dt.float32, tag=tag)
                nc.tensor.matmul(ps[:], wsb[wname], rhs_ap, start=True, stop=True)
                return ps

            # AdaLN for attention branch
            g_ps = mm("w_gate", sn_T[:], NW, "mm")
            gate_sb = stmp.tile([128, NW], dt.bfloat16, tag="gate")
            nc.scalar.activation(gate_sb[:], g_ps[:], AF.Sigmoid, bias=vsb["gate_b"])
            sk_ps = mm("w_skip", sn_T[:], NW, "mm")
            tg1 = stmp.tile([128, NW], dt.bfloat16, tag="atg1")
            nc.vector.tensor_tensor(tg1[:], gate_sb[:], lna_T[:], op=ALU.mult)
            a_T = sing.tile([128, NW], dt.bfloat16, tag="a_T")
            nc.vector.tensor_tensor(a_T[:], tg1[:], sk_ps[:], op=ALU.add)

            q_Ts, k_Ts = [], []
            for grp in ("a", "b"):
                q_ps = mm(f"wq_{grp}", a_T[:, qsl], NQ, "mm")
                q_Tg = sing.tile([128, NQ], dt.bfloat16, tag=f"q_T{grp}")
                nc.scalar.copy(q_Tg[:], q_ps[:])  # D^-0.5 folded into wq
                q_Ts.append(q_Tg)
                k_ps = mm(f"wk_{grp}", a_T[:], NW, "mm")
                k_Tg = sing.tile([128, NW], dt.bfloat16, tag=f"k_T{grp}")
                nc.scalar.copy(k_Tg[:], k_ps[:])
                k_Ts.append(k_Tg)
            v_ps = mm("wv", a_T[:], NW, "mm")
            v_T = sing.tile([128, NW], dt.bfloat16, tag="v_T")
            nc.scalar.copy(v_T[:], v_ps[:])
            gq_Ts = []
            for grp in ("a", "b"):
                g2_ps = mm(f"wg_{grp}", a_T[:, qsl], NQ, "mm")
                gq_Tg = sing.tile([128, NQ], dt.bfloat16, tag=f"gq_T{grp}")
                nc.scalar.activation(
                    gq_Tg[:], g2_ps[:], AF.Sigmoid, bias=vsb[f"bg_{grp}"]
                )
                gq_Ts.append(gq_Tg)

            # V_ext per w-chunk: per head 48 cols: [ones | 31 zero | V_h(16)]
            VE = 48
            v_ext = []
            for wc in range(NWC):
                tp = trgn.tile([128, 128], dt.bfloat16, tag="rgn")
                nc.tensor.transpose(tp[:], v_T[:, wc * 128 : (wc + 1) * 128], ident[:])
                ve = sing.tile([128, H * VE], dt.bfloat16, tag=f"vext{wc}")  # 1-buf pool: unique tags
                nc.vector.memset(ve[:], 0.0)
                veb, tpb = ve[:], tp[:]
                ones_ap = bass.AP(
                    tensor=veb.tensor, offset=veb.offset,
                    ap=[veb.ap[0], [VE, H], [1, D]],
                )
                nc.vector.memset(ones_ap, 1.0)
                dst = bass.AP(
                    tensor=veb.tensor, offset=veb.offset + 32,
                    ap=[veb.ap[0], [VE, H], [1, D]],
                )
                src = bass.AP(
                    tensor=tpb.tensor, offset=tpb.offset,
                    ap=[tpb.ap[0], [D, H], [1, D]],
                )
                nc.vector.tensor_copy(dst, src)
                v_ext.append(ve)

            # out gates (raw sp projections)
            og_ps = mm("w_og", sp_T[:, qsl], NQ, "mm")
            og_sb = stmp.tile([128, NQ], dt.bfloat16, tag="og")
            nc.scalar.activation(og_sb[:], og_ps[:], AF.Sigmoid, bias=vsb["og_b"])
            tog_ps = mm("w_tog", sp_T[:, qsl], NQ, "mm")
            tog_sb = stmp.tile([128, NQ], dt.bfloat16, tag="tog")
            nc.scalar.activation(tog_sb[:], tog_ps[:], AF.Sigmoid, bias=vsb["tog_b"])

            # transition branch
            tgp = mm("w_tgate", sn_T[:, qsl], NQ, "mm")
            tgate = stmp.tile([128, NQ], dt.bfloat16, tag="tgate")
            nc.scalar.activation(tgate[:], tgp[:], AF.Sigmoid, bias=vsb["t_gate_b"])
            tskp = mm("w_tskip", sn_T[:, qsl], NQ, "mm")
            tt1 = stmp.tile([128, NQ], dt.bfloat16, tag="tt1")
            nc.vector.tensor_tensor(tt1[:], tgate[:], lna_T[:, qsl], op=ALU.mult)
            t_T = sing.tile([128, NQ], dt.bfloat16, tag="t_T")
            nc.vector.tensor_tensor(t_T[:], tt1[:], tskp[:], op=ALU.add)

            hid = []
            for half in range(2):
                swp = mm(f"w_sw{half}", t_T[:], NQ, "mm")
                sw = stmp.tile([128, NQ], dt.bfloat16, tag=f"sw{half}")
                nc.scalar.activation(sw[:], swp[:], AF.Silu)
                hdp = mm(f"w_hd{half}", t_T[:], NQ, "mm")
                hh = sing.tile([128, NQ], dt.bfloat16, tag=f"hid{half}")
                nc.vector.tensor_tensor(hh[:], sw[:], hdp[:], op=ALU.mult)
                hid.append(hh)
            tr_ps = spsum.tile([128, NQ], dt.float32, tag="mm")
            nc.tensor.matmul(tr_ps[:], wsb["w_to0"], hid[0][:], start=True, stop=False)
            nc.tensor.matmul(tr_ps[:], wsb["w_to1"], hid[1][:], start=False, stop=True)
            trans_g = sing.tile([128, NQ], dt.float32, tag="trans_g")
            nc.vector.tensor_tensor(trans_g[:], tog_sb[:], tr_ps[:], op=ALU.mult)

            # ============ pair path ============
            # LN stats: mean and rstd per (w, q). Mean-subtraction of the bias
            # itself is folded into wblk on host; m only feeds the variance.
            bacc_t = sbig.tile([128, H * NWC * NQ], dt.bfloat16, tag="bacc")
            rs_bs = []
            for wc in range(NWC):
                pwt = pw_tiles[wc]
                pw3 = pwt[:].rearrange("p (q c) -> p q c", c=CP)
                sums = stmp.tile([128, NQ], dt.bfloat16, tag="sums")
                nc.vector.tensor_reduce(out=sums[:], in_=pw3, axis=AX.X, op=ALU.add)
                sq = sq_tiles[wc]
                sumsq = stmp.tile([128, NQ], dt.bfloat16, tag="sumsq")
                nc.vector.tensor_reduce(
                    out=sumsq[:], in_=sq[:].rearrange("p (q c) -> p q c", c=CP),
                    axis=AX.X, op=ALU.add,
                )
                m = stmp.tile([128, NQ], dt.float32, tag="m")
                nc.vector.tensor_scalar_mul(m[:], sums[:], 1.0 / CP)
                nm2 = stmp.tile([128, NQ], dt.float32, tag="nm2")
                nc.vector.scalar_tensor_tensor(
                    nm2[:], m[:], -1.0, m[:], ALU.mult, ALU.mult
                )
                var = stmp.tile([128, NQ], dt.float32, tag="var")
                nc.vector.scalar_tensor_tensor(
                    var[:], sumsq[:], 1.0 / CP, nm2[:], ALU.mult, ALU.add
                )
                varc = stmp.tile([128, NQ], dt.float32, tag="varc")
                nc.vector.tensor_scalar_max(varc[:], var[:], 0.0)
                sd = stmp.tile([128, NQ], dt.float32, tag="sdp")
                nc.scalar.activation(sd[:], varc[:], AF.Sqrt, bias=EPS)
                rs = stmp.tile([128, NQ], dt.float32, tag="rsp")
                nc.vector.reciprocal_approx_fast(rs[:], sd[:])
                rs_b = stmp.tile([128, NQ], dt.bfloat16, tag="rsb")
                nc.vector.tensor_copy(rs_b[:], rs[:])
                rs_bs.append(rs_b)

                # packed z matmuls -> Z [128=(qm,h), qG*128+w]
                pka = pka_tiles[wc]
                pkb = pkb_tiles[wc]
                z_sb = zsb.tile([128, NQG * 128], dt.bfloat16, tag="z_sb")
                for ch in range(4):
                    csl = slice(ch * 512, (ch + 1) * 512)
                    zp = spsum.tile([128, 512], dt.float32, tag="mm")
                    nc.tensor.matmul(zp[:], wsb["wblk_a"], pka[:, csl], start=True, stop=False)
                    nc.tensor.matmul(zp[:], wsb["wblk_b"], pkb[:, csl], start=False, stop=True)
                    if ch % 2 == 0:
                        nc.scalar.copy(z_sb[:, csl], zp[:])
                    else:
                        nc.vector.tensor_copy(z_sb[:, csl], zp[:])

                # transpose 4-chunk regions, then one strided scale per region:
                # bacc[h*768 + wc*256 + qg*16 + r] = zT[(r,h)] * rs[q]
                rs_bb = rs_b[:]
                bab = bacc_t[:]
                for rg in range(4):
                    rgn = trgn.tile([128, 512], dt.bfloat16, tag="rgn")
                    for k in range(4):
                        qg = rg * 4 + k
                        nc.tensor.transpose(
                            rgn[:, k * 128 : (k + 1) * 128],
                            z_sb[:, qg * 128 : (qg + 1) * 128], ident[:],
                        )
                    rgb = rgn[:]
                    zread = bass.AP(
                        tensor=rgb.tensor, offset=rgb.offset,
                        ap=[rgb.ap[0], [1, H], [H, 64]],
                    )
                    rsrd = bass.AP(
                        tensor=rs_bb.tensor, offset=rs_bb.offset + rg * 64,
                        ap=[rs_bb.ap[0], [0, H], [1, 64]],
                    )
                    dst = bass.AP(
                        tensor=bab.tensor,
                        offset=bab.offset + wc * NQ + rg * 64,
                        ap=[bab.ap[0], [NWC * NQ, H], [1, 64]],
                    )
                    nc.vector.tensor_tensor(dst, zread, rsrd, op=ALU.mult)

            # ============ attention ============
            t2g = []
            for grp in range(2):
                t = sing.tile([128, NQ], dt.bfloat16, tag=f"t2g{grp}")
                nc.vector.memset(t[:], 0.0)
                t2g.append(t)
            for h in range(H):
                grp, hb = h // 4, (h % 4) * 32
                h32 = slice(hb, hb + 32)
                s_big = sbp.tile([128, NWC * NQ], dt.float32, tag="sbig")
                for wc in range(NWC):
                    wsl = slice(wc * 128, (wc + 1) * 128)
                    qslw = slice(wc * NQ, (wc + 1) * NQ)
                    # in-window query range for this w-chunk (rest masked)
                    qv = (slice(0, 128), slice(0, NQ), slice(128, NQ))[wc]
                    nc.tensor.matmul(
                        s_big[:, qslw], ident[:],
                        bacc_t[:, h * NWC * NQ + wc * NQ : h * NWC * NQ + (wc + 1) * NQ],
                        start=True, stop=False, skip_group_check=True,
                    )
                    nc.tensor.matmul(
                        s_big[:, qslw], ident[:], wa_cat[:, qslw],
                        start=False, stop=False, skip_group_check=True,
                    )
                    nc.tensor.matmul(
                        s_big[:, wc * NQ + qv.start : wc * NQ + qv.stop],
                        k_Ts[grp][h32, wsl], q_Ts[grp][h32, qv],
                        start=False, stop=True, tile_position=(hb, 0),
                        skip_group_check=True,
                    )
                p_big = stmp.tile([128, NWC * NQ], dt.bfloat16, tag="p_big")
                nc.scalar.activation(
                    p_big[:], s_big[:], AF.Exp, bias=c0_sb[:, h : h + 1]
                )
                ov = spsum.tile([VE, NQ], dt.float32, tag="mm")
                for wc in range(NWC):
                    nc.tensor.matmul(
                        ov[:], v_ext[wc][:, h * VE : (h + 1) * VE],
                        p_big[:, wc * NQ : (wc + 1) * NQ],
                        start=(wc == 0), stop=(wc == NWC - 1),
                    )
                zb_f = stmp.tile([D, NQ], dt.float32, tag="zb_f")
                nc.vector.reciprocal_approx_fast(zb_f[:], ov[0:D, :])
                u = stmp.tile([128, NQ], dt.bfloat16, tag="u")
                nc.vector.tensor_tensor(
                    u[hb : hb + D, :], zb_f[:], ov[32 : 32 + D, :], op=ALU.mult
                )
                nc.vector.tensor_tensor(
                    t2g[h // 4][hb : hb + D, :], u[hb : hb + D, :],
                    gq_Ts[h // 4][hb : hb + D, :], op=ALU.mult,
                )

            ao_ps = spsum.tile([128, NQ], dt.float32, tag="mm")
            nc.tensor.matmul(ao_ps[:], wsb["wo_a"], t2g[0][:], start=True, stop=False)
            nc.tensor.matmul(ao_ps[:], wsb["wo_b"], t2g[1][:], start=False, stop=True)
            attn = stmp.tile([128, NQ], dt.float32, tag="attn")
            nc.vector.scalar_tensor_tensor(
                attn[:], ao_ps[:], vsb["bo"], og_sb[:], ALU.add, ALU.mult
            )
            fin = stmp.tile([128, NQ], dt.float32, tag="fin")
            nc.vector.tensor_tensor(fin[:], attn[:], trans_g[:], op=ALU.add)

            # transpose [C, q] -> [q, C] and write out
            for qt in range(2):
                qts = slice(qt * 128, (qt + 1) * 128)
                otp = trgn.tile([128, 128], dt.float32, tag="rgn")
                nc.tensor.transpose(otp[:], fin[:, qts], ident_f[:])
                osb = stmp.tile([128, 128], dt.float32, tag="osb")
                nc.scalar.copy(osb[:], otp[:])
                nc.sync.dma_start(out=out_ext[qts, :], in_=osb[:])

    nc.finalize()
    return nc


def _prep(inputs):
    """Host-side shard + pack. Returns (in_maps, consts)."""
    f = {k: np.asarray(v) for k, v in inputs.items()}
    pair = f["atom_pair_repr"][0]          # [N, N, CP]
    asr = f["atom_single_repr"][0]         # [N, C]
    sp = f["atom_single_proj"][0]
    mask = f["mask"][0]                    # [N]

    ln_scale = f["pair_ln_scale"]
    ln_bias = f["pair_ln_bias"]
    wp_s = ln_scale[:, None] * f["w_pair"]           # [CP, H] scaled
    c0 = (ln_bias @ f["w_pair"]).astype(np.float64)  # [H]
    wp = wp_s - wp_s.sum(0, keepdims=True) / CP      # fold mean subtraction

    s_scale = f["adaln_s_scale"]
    t_scale = f["t_s_scale"]

    def bf(x):
        return np.ascontiguousarray(x.astype(BF16))

    def pad_heads(w, scale=1.0):
        a = np.zeros((C, 128), F32)
        b = np.zeros((C, 128), F32)
        for h4 in range(4):
            a[:, h4 * 32 : h4 * 32 + D] = w[:, h4 * D : (h4 + 1) * D] * scale
            b[:, h4 * 32 : h4 * 32 + D] = w[:, (h4 + 4) * D : (h4 + 5) * D] * scale
        return bf(a), bf(b)

    wq_a, wq_b = pad_heads(f["wq"], float(D) ** -0.5)
    wk_a, wk_b = pad_heads(f["wk"])
    wg_a, wg_b = pad_heads(f["wg"])

    def pad_rows(w):
        a = np.zeros((128, C), F32)
        b = np.zeros((128, C), F32)
        for h4 in range(4):
            a[h4 * 32 : h4 * 32 + D, :] = w[h4 * D : (h4 + 1) * D, :]
            b[h4 * 32 : h4 * 32 + D, :] = w[(h4 + 4) * D : (h4 + 5) * D, :]
        return bf(a), bf(b)

    wo_a, wo_b = pad_rows(f["wo"])
    bg_pad = np.zeros((2, 128), F32)
    for h4 in range(4):
        bg_pad[0, h4 * 32 : h4 * 32 + D] = f["bg"][h4 * D : (h4 + 1) * D]
        bg_pad[1, h4 * 32 : h4 * 32 + D] = f["bg"][(h4 + 4) * D : (h4 + 5) * D]

    wblk_a = np.zeros((128, 128), F32)
    wblk_b = np.zeros((128, 128), F32)
    for qm in range(8):
        wblk_a[qm * 16 : qm * 16 + 16, qm * 8 : qm * 8 + 8] = wp
        wblk_b[qm * 16 : qm * 16 + 16, 64 + qm * 8 : 64 + qm * 8 + 8] = wp

    weights = {
        "wq_a": wq_a, "wq_b": wq_b, "wk_a": wk_a, "wk_b": wk_b,
        "wg_a": wg_a, "wg_b": wg_b, "wo_a": wo_a, "wo_b": wo_b,
        "w_gate": bf(s_scale[:, None] * f["adaln_gate_w"]),
        "w_skip": bf(s_scale[:, None] * f["adaln_skip_w"]),
        "wv": bf(f["wv"]),
        "w_og": bf(f["out_gate_w"]),
        "w_tgate": bf(t_scale[:, None] * f["t_gate_w"]),
        "w_tskip": bf(t_scale[:, None] * f["t_skip_w"]),
        "w_sw0": bf(f["t_swish_w"][:, :128]), "w_sw1": bf(f["t_swish_w"][:, 128:]),
        "w_hd0": bf(f["t_hidden_w"][:, :128]), "w_hd1": bf(f["t_hidden_w"][:, 128:]),
        "w_to0": bf(f["t_out_w"][:128, :]), "w_to1": bf(f["t_out_w"][128:, :]),
        "w_tog": bf(f["t_out_gate_w"]),
        "wblk_a": bf(wblk_a), "wblk_b": bf(wblk_b),
    }
    wcat = np.concatenate([weights[k] for k in WNAMES], axis=1)
    vecs = {
        "gate_b": f["adaln_gate_b"], "bg_a": bg_pad[0], "bg_b": bg_pad[1],
        "bo": f["bo"],
        "og_b": f["out_gate_b"], "t_gate_b": f["t_gate_b"],
        "tog_b": f["t_out_gate_b"],
    }
    vcat = np.stack(
        [vecs[k].astype(F32).reshape(128) for k in VNAMES], axis=1
    )

    shared = {
        "wcat": np.ascontiguousarray(wcat),
        "vcat": np.ascontiguousarray(vcat),
    }

    in_maps = []
    qidx = np.arange(NQ)
    for i in range(NCORES):
        i0 = i * NQ
        lo = i0 - WH
        ks, ke = max(lo, 0), min(i0 + NQ + WH, N)
        strip = np.zeros((NQ, NW, CP), F32)
        strip[:, ks - lo : ke - lo] = pair[i0 : i0 + NQ, ks:ke]
        halo = np.zeros((NW, C), F32)
        halo_s = np.zeros((NW, C), F32)
        halo[ks - lo : ke - lo] = asr[ks:ke]
        halo_s[ks - lo : ke - lo] = sp[ks:ke]
        scat = np.concatenate(
            [x for rt in range(NWC)
             for x in (halo_s[rt * 128 : (rt + 1) * 128],
                       halo[rt * 128 : (rt + 1) * 128])],
            axis=1,
        )

        pw = strip.transpose(1, 0, 2).reshape(NW, NQ * CP)
        pair_w_i = bf(pw.reshape(NWC, 128, NQ * CP))

        s4 = strip.reshape(NQG, 16, NW, CP)          # [qG, qm, w, cp]
        def pack(half):
            t = s4[:, half * 8 : half * 8 + 8]       # [qG, 8, w, cp]
            t = t.transpose(1, 3, 0, 2)              # [qm, cp, qG, w]
            t = t.reshape(128, NQG, NWC, 128)        # [p, qG, wc, wl]
            return bf(t.transpose(2, 0, 1, 3).reshape(NWC, 128, NQG * 128))
        pk_a_i = pack(0)
        pk_b_i = pack(1)

        kabs = lo + np.arange(NW)
        inb = (kabs >= 0) & (kabs < N)
        mstrip = np.where(inb, mask[np.clip(kabs, 0, N - 1)], 0.0)
        valid = (
            (np.abs(kabs[:, None] - (i0 + qidx)[None, :]) <= WH)
            & inb[:, None]
            & (mstrip[:, None] > 0.5)
        )
        wa = np.where(valid, 0.0, NEG).reshape(NWC, 128, NQ)
        winadd_i = bf(wa.transpose(1, 0, 2).reshape(128, NWC * NQ))

        in_maps.append({
            **shared,
            "scat": np.ascontiguousarray(scat),
            "pair_w": pair_w_i,
            "pk_a": pk_a_i,
            "pk_b": pk_b_i,
            "winadd": winadd_i,
        })
    return in_maps, {"c0": c0}


_CACHE = {}


def kernel(**inputs):
    in_maps, consts = _prep(inputs)
    key = "graph"
    if key not in _CACHE:
        _CACHE[key] = build_graph(consts)
    nc = _CACHE[key]
    res = run_bass_kernel_spmd(nc, in_maps, core_ids=list(range(NCORES)))
    out = np.concatenate([res.results[i]["out"] for i in range(NCORES)], axis=0)
    return out.reshape(1, N, C).astype(np.float32)


if __name__ == "__main__":
    import reference

    ins = reference.setup_inputs()
    ins = {k: np.asarray(v) for k, v in ins.items()}
    got = kernel(**ins)
    exp = np.asarray(reference.reference(**reference.setup_inputs()))
    err = np.abs(got - exp).max() / (np.abs(exp).max() + 1e-9)
    print("Relative error:", err)


# revision 13
# speedup vs baseline: 1.2244x; 1.2244x over previous
"""Trainium2 Bass kernel for AtomTransformerBlock (sliding-window attention, W=64).

Sharding: 8 cores x 256 contiguous query atoms (sequence parallel), each core
gets a 64-atom halo of the singles and the corresponding pair-bias band strip.
No collectives needed (halo recompute). All heavy compute in bf16.

v2: packed weight/vec/singles DMAs, full pair prefetch, mean-subtraction folded
into the pair projection weights on host (bias = rs*(pair@wp') + c0), batched
bias-transpose scaling, winadd folded into the bias tile.
"""

import sys

sys.path.insert(0, "/opt/trn_rl_repo")

import numpy as np
import ml_dtypes

import concourse.bass as bass
import concourse.tile as tile
from concourse import bacc, masks, mybir
from concourse.bass_utils import run_bass_kernel_spmd

BF16 = ml_dtypes.bfloat16
F32 = np.float32

N = 2048
C = 128
CP = 16
H = 8
D = 16
WH = 64
NCORES = 8
NQ = N // NCORES          # 256 queries per core
NW = NQ + 2 * WH          # 384 window positions per core
NWC = NW // 128           # 3 w-chunks
NQG = NQ // 16            # 16 query groups of 16
EPS = 1e-5
NEG = -30000.0

dt = mybir.dt
AF = mybir.ActivationFunctionType
ALU = mybir.AluOpType
AX = mybir.AxisListType

WNAMES = [
    "w_gate", "w_skip", "wq_a", "wq_b", "wk_a", "wk_b", "wv",
    "wg_a", "wg_b", "wo_a", "wo_b", "w_og",
    "w_tgate", "w_tskip", "w_sw0", "w_sw1", "w_hd0", "w_hd1",
    "w_to0", "w_to1", "w_tog", "wblk_a", "wblk_b",
]
VNAMES = ["gate_b", "bg_a", "bg_b", "bo", "og_b", "t_gate_b", "tog_b"]


def build_graph(consts, skip=()):
    nc = bacc.Bacc(
        "TRN2",
        target_bir_lowering=False,
        debug=False,
        enable_asserts=False,
        num_devices=NCORES,
    )

    def inp(name, shape, dtype):
        return nc.dram_tensor(name, shape, dtype, kind="ExternalInput")

    # Per-core data (packed)
    scat_d = inp("scat", [128, 6 * 128], dt.float32)       # (sp,asr) x 3 row-tiles
    pair_w = inp("pair_w", [NWC, 128, NQ * CP], dt.bfloat16)
    pk_a = inp("pk_a", [NWC, 128, NQG * 128], dt.bfloat16)
    pk_b = inp("pk_b", [NWC, 128, NQG * 128], dt.bfloat16)
    winadd = inp("winadd", [128, NWC * NQ], dt.bfloat16)
    wcat_d = inp("wcat", [128, len(WNAMES) * 128], dt.bfloat16)
    vcat_d = inp("vcat", [128, len(VNAMES)], dt.float32)

    out_ext = nc.dram_tensor("out", [NQ, C], dt.float32, kind="ExternalOutput")

    c0 = consts["c0"]  # [H] python floats

    with tile.TileContext(nc) as tc:
        with (
            nc.allow_low_precision(reason="bf16 compute; tolerance 2e-2"),
            tc.tile_pool(name="const", bufs=1) as constp,
            tc.tile_pool(name="wpool", bufs=1) as wpool,
            tc.tile_pool(name="sing", bufs=1) as sing,
            tc.tile_pool(name="sbig", bufs=1) as sbig,
            tc.tile_pool(name="stmp", bufs=3) as stmp,
            tc.tile_pool(name="pairp", bufs=3) as pairp,
            tc.tile_pool(name="sqp", bufs=3) as sqp,
            tc.tile_pool(name="zsb", bufs=3) as zsb,
            tc.tile_pool(name="trgn", bufs=2, space="PSUM") as trgn,
            tc.tile_pool(name="spsum", bufs=2, space="PSUM") as spsum,
            tc.tile_pool(name="sbp", bufs=2, space="PSUM") as sbp,
        ):
            # --- packed input loads (few, large DMAs) ---
            scat = wpool.tile([128, 6 * 128], dt.float32, tag="scat")
            nc.sync.dma_start(out=scat[:], in_=scat_d[:])
            wcat = wpool.tile([128, len(WNAMES) * 128], dt.bfloat16, tag="wcat")
            nc.sync.dma_start(out=wcat[:], in_=wcat_d[:])
            vcat = wpool.tile([128, len(VNAMES)], dt.float32, tag="vcat")
            nc.sync.dma_start(out=vcat[:], in_=vcat_d[:])
            wa_cat = wpool.tile([128, NWC * NQ], dt.bfloat16, tag="wa_cat")
            nc.sync.dma_start(out=wa_cat[:], in_=winadd[:])

            # prefetch ALL pair data immediately (dedicated bufs => no gating)
            pw_tiles, pka_tiles, pkb_tiles = [], [], []
            for wc in range(NWC):
                pka = pairp.tile([128, NQG * 128], dt.bfloat16, tag="pka")
                nc.sync.dma_start(out=pka[:], in_=pk_a[wc])
                pka_tiles.append(pka)
                pkb = pairp.tile([128, NQG * 128], dt.bfloat16, tag="pkb")
                nc.sync.dma_start(out=pkb[:], in_=pk_b[wc])
                pkb_tiles.append(pkb)
            for wc in range(NWC):
                pwt = pairp.tile([128, NQ * CP], dt.bfloat16, tag="pair_w")
                nc.sync.dma_start(out=pwt[:], in_=pair_w[wc])
                pw_tiles.append(pwt)

            # --- constants ---
            zero_c = constp.tile([128, 1], dt.float32, tag="zero_c")
            nc.vector.memset(zero_c[:], 0.0)
            eps_c = constp.tile([128, 1], dt.float32, tag="eps_c")
            nc.vector.memset(eps_c[:], EPS)
            nc.const_aps.aps[(dt.float32, 0.0)] = zero_c[:]
            nc.const_aps.aps[(dt.float32, EPS)] = eps_c[:]
            ident = constp.tile([128, 128], dt.bfloat16)
            masks.make_identity(nc, ident[:])
            ident_f = constp.tile([128, 128], dt.float32, tag="ident_f")
            masks.make_identity(nc, ident_f[:])

            wsb = {k: wcat[:, i * 128 : (i + 1) * 128] for i, k in enumerate(WNAMES)}
            vsb = {k: vcat[:, i : i + 1] for i, k in enumerate(VNAMES)}
            c0_sb = wpool.tile([128, H], dt.float32, tag="c0")
            for h in range(H):
                nc.gpsimd.memset(c0_sb[:, h : h + 1], float(c0[h]))

            # ============ singles path ============
            # rows-on-partition LN, then PE-transpose into [C, rows] chain layout
            sn_T = sing.tile([128, NW], dt.bfloat16, tag="sn_T")
            lna_T = sing.tile([128, NW], dt.bfloat16, tag="lna_T")
            sp_T = sing.tile([128, NW], dt.bfloat16, tag="sp_T")
            for rt in range(NWC):
                rows = slice(rt * 128, (rt + 1) * 128)
                for ti, (nm, dstT, do_raw) in enumerate(
                    (("sp", sn_T, True), ("asr", lna_T, False))
                ):
                    xt = scat[:, (rt * 2 + ti) * 128 : (rt * 2 + ti + 1) * 128]
                    stats = stmp.tile([128, 6], dt.float32, tag="ln_stats")
                    nc.vector.bn_stats(out=stats[:], in_=xt)
                    mv = stmp.tile([128, 2], dt.float32, tag="ln_mv")
                    nc.vector.bn_aggr(out=mv[:], in_=stats[:])
                    sd = stmp.tile([128, 1], dt.float32, tag="ln_sd")
                    nc.scalar.activation(sd[:], mv[:, 1:2], AF.Sqrt, bias=EPS)
                    rsv = stmp.tile([128, 1], dt.float32, tag="ln_rs")
                    nc.vector.reciprocal_approx_fast(rsv[:], sd[:])
                    nmrs = stmp.tile([128, 1], dt.float32, tag="ln_nm")
                    nc.vector.scalar_tensor_tensor(
                        nmrs[:], mv[:, 0:1], -1.0, rsv[:], ALU.mult, ALU.mult
                    )
                    lnx = stmp.tile([128, 128], dt.bfloat16, tag="ln_out")
                    nc.scalar.activation(
                        lnx[:], xt, AF.Identity, bias=nmrs[:], scale=rsv[:]
                    )
                    tp = trgn.tile([128, 128], dt.bfloat16, tag="rgn")
                    nc.tensor.transpose(tp[:], lnx[:], ident[:])
                    nc.scalar.copy(dstT[:, rows], tp[:])
                    if do_raw:
                        tpr = trgn.tile([128, 128], dt.bfloat16, tag="rgn")
                        xb = stmp.tile([128, 128], dt.bfloat16, tag="ln_xb")
                        nc.scalar.copy(xb[:], xt)
                        nc.tensor.transpose(tpr[:], xb[:], ident[:])
                        nc.scalar.copy(sp_T[:, rows], tpr[:])

            # squares of pair (scalar engine for wc0/2, gpsimd wc1: keep DVE free)
            sq_tiles = []
            for wc in range(NWC):
                sq = sqp.tile([128, NQ * CP], dt.bfloat16, tag="sq")
                if wc == 1:
                    nc.gpsimd.tensor_tensor(
                        sq[:], pw_tiles[wc][:], pw_tiles[wc][:], op=ALU.mult
                    )
                else:
                    nc.scalar.square(sq[:], pw_tiles[wc][:])
                sq_tiles.append(sq)
            qsl = slice(WH, WH + NQ)  # center 256 of the 384 halo rows

            def mm(wname, rhs_ap, n, tag):
                ps = spsum.tile([128, n], dt.float32, tag=tag)
                nc.tensor.matmul(ps[:], wsb[wname], rhs_ap, start=True, stop=True)
                return ps

            # AdaLN for attention branch
            g_ps = mm("w_gate", sn_T[:], NW, "mm")
            gate_sb = stmp.tile([128, NW], dt.bfloat16, tag="gate")
            nc.scalar.activation(gate_sb[:], g_ps[:], AF.Sigmoid, bias=vsb["gate_b"])
            sk_ps = mm("w_skip", sn_T[:], NW, "mm")
            tg1 = stmp.tile([128, NW], dt.bfloat16, tag="atg1")
            nc.vector.tensor_tensor(tg1[:], gate_sb[:], lna_T[:], op=ALU.mult)
            a_T = sing.tile([128, NW], dt.bfloat16, tag="a_T")
            nc.vector.tensor_tensor(a_T[:], tg1[:], sk_ps[:], op=ALU.add)

            q_Ts, k_Ts = [], []
            for grp in ("a", "b"):
                q_ps = mm(f"wq_{grp}", a_T[:, qsl], NQ, "mm")
                q_Tg = sing.tile([128, NQ], dt.bfloat16, tag=f"q_T{grp}")
                nc.scalar.copy(q_Tg[:], q_ps[:])  # D^-0.5 folded into wq
                q_Ts.append(q_Tg)
                k_ps = mm(f"wk_{grp}", a_T[:], NW, "mm")
                k_Tg = sing.tile([128, NW], dt.bfloat16, tag=f"k_T{grp}")
                nc.scalar.copy(k_Tg[:], k_ps[:])
                k_Ts.append(k_Tg)
            v_ps = mm("wv", a_T[:], NW, "mm")
            v_T = sing.tile([128, NW], dt.bfloat16, tag="v_T")
            nc.scalar.copy(v_T[:], v_ps[:])
            gq_Ts = []
            for grp in ("a", "b"):
                g2_ps = mm(f"wg_{grp}", a_T[:, qsl], NQ, "mm")
                gq_Tg = sing.tile([128, NQ], dt.bfloat16, tag=f"gq_T{grp}")
                nc.scalar.activation(
                    gq_Tg[:], g2_ps[:], AF.Sigmoid, bias=vsb[f"bg_{grp}"]
                )
                gq_Ts.append(gq_Tg)

            # V_ext per w-chunk: per head 48 cols: [ones | 31 zero | V_h(16)]
            VE = 48
            v_ext = []
            for wc in range(NWC):
                tp = trgn.tile([128, 128], dt.bfloat16, tag="rgn")
                nc.tensor.transpose(tp[:], v_T[:, wc * 128 : (wc + 1) * 128], ident[:])
                ve = sing.tile([128, H * VE], dt.bfloat16, tag=f"vext{wc}")  # 1-buf pool: unique tags
                nc.vector.memset(ve[:], 0.0)
                veb, tpb = ve[:], tp[:]
                ones_ap = bass.AP(
                    tensor=veb.tensor, offset=veb.offset,
                    ap=[veb.ap[0], [VE, H], [1, D]],
                )
                nc.vector.memset(ones_ap, 1.0)
                dst = bass.AP(
                    tensor=veb.tensor, offset=veb.offset + 32,
                    ap=[veb.ap[0], [VE, H], [1, D]],
                )
                src = bass.AP(
                    tensor=tpb.tensor, offset=tpb.offset,
                    ap=[tpb.ap[0], [D, H], [1, D]],
                )
                nc.vector.tensor_copy(dst, src)
                v_ext.append(ve)

            # out gates (raw sp projections)
            og_ps = mm("w_og", sp_T[:, qsl], NQ, "mm")
            og_sb = stmp.tile([128, NQ], dt.bfloat16, tag="og")
            nc.scalar.activation(og_sb[:], og_ps[:], AF.Sigmoid, bias=vsb["og_b"])
            tog_ps = mm("w_tog", sp_T[:, qsl], NQ, "mm")
            tog_sb = stmp.tile([128, NQ], dt.bfloat16, tag="tog")
            nc.scalar.activation(tog_sb[:], tog_ps[:], AF.Sigmoid, bias=vsb["tog_b"])

            # transition branch
            tgp = mm("w_tgate", sn_T[:, qsl], NQ, "mm")
            tgate = stmp.tile([128, NQ], dt.bfloat16, tag="tgate")
            nc.scalar.activation(tgate[:], tgp[:], AF.Sigmoid, bias=vsb["t_gate_b"])
            tskp = mm("w_tskip", sn_T[:, qsl], NQ, "mm")
            tt1 = stmp.tile([128, NQ], dt.bfloat16, tag="tt1")
            nc.vector.tensor_tensor(tt1[:], tgate[:], lna_T[:, qsl], op=ALU.mult)
            t_T = sing.tile([128, NQ], dt.bfloat16, tag="t_T")
            nc.vector.tensor_tensor(t_T[:], tt1[:], tskp[:], op=ALU.add)

            hid = []
            for half in range(2):
                swp = mm(f"w_sw{half}", t_T[:], NQ, "mm")
                sw = stmp.tile([128, NQ], dt.bfloat16, tag=f"sw{half}")
                nc.scalar.activation(sw[:], swp[:], AF.Silu)
                hdp = mm(f"w_hd{half}", t_T[:], NQ, "mm")
                hh = sing.tile([128, NQ], dt.bfloat16, tag=f"hid{half}")
                nc.vector.tensor_tensor(hh[:], sw[:], hdp[:], op=ALU.mult)
                hid.append(hh)
            tr_ps = spsum.tile([128, NQ], dt.float32, tag="mm")
            nc.tensor.matmul(tr_ps[:], wsb["w_to0"], hid[0][:], start=True, stop=False)
            nc.tensor.matmul(tr_ps[:], wsb["w_to1"], hid[1][:], start=False, stop=True)
            trans_g = sing.tile([128, NQ], dt.float32, tag="trans_g")
            nc.vector.tensor_tensor(trans_g[:], tog_sb[:], tr_ps[:], op=ALU.mult)

            # ============ pair path ============
            # LN stats: mean and rstd per (w, q). Mean-subtraction of the bias
            # itself is folded into wblk on host; m only feeds the variance.
            bacc_t = sbig.tile([128, H * NWC * NQ], dt.bfloat16, tag="bacc")
            rs_bs = []
            for wc in range(NWC):
                pwt = pw_tiles[wc]
                pw3 = pwt[:].rearrange("p (q c) -> p q c", c=CP)
                sums = stmp.tile([128, NQ], dt.bfloat16, tag="sums")
                nc.vector.tensor_reduce(out=sums[:], in_=pw3, axis=AX.X, op=ALU.add)
                sq = sq_tiles[wc]
                sumsq = stmp.tile([128, NQ], dt.bfloat16, tag="sumsq")
                nc.vector.tensor_reduce(
                    out=sumsq[:], in_=sq[:].rearrange("p (q c) -> p q c", c=CP),
                    axis=AX.X, op=ALU.add,
                )
                m = stmp.tile([128, NQ], dt.float32, tag="m")
                nc.vector.tensor_scalar_mul(m[:], sums[:], 1.0 / CP)
                nm2 = stmp.tile([128, NQ], dt.float32, tag="nm2")
                nc.vector.scalar_tensor_tensor(
                    nm2[:], m[:], -1.0, m[:], ALU.mult, ALU.mult
                )
                var = stmp.tile([128, NQ], dt.float32, tag="var")
                nc.vector.scalar_tensor_tensor(
                    var[:], sumsq[:], 1.0 / CP, nm2[:], ALU.mult, ALU.add
                )
                varc = stmp.tile([128, NQ], dt.float32, tag="varc")
                nc.vector.tensor_scalar_max(varc[:], var[:], 0.0)
                sd = stmp.tile([128, NQ], dt.float32, tag="sdp")
                nc.scalar.activation(sd[:], varc[:], AF.Sqrt, bias=EPS)
                rs = stmp.tile([128, NQ], dt.float32, tag="rsp")
                nc.vector.reciprocal_approx_fast(rs[:], sd[:])
                rs_b = stmp.tile([128, NQ], dt.bfloat16, tag="rsb")
                nc.scalar.copy(rs_b[:], rs[:])
                rs_bs.append(rs_b)

                # packed z matmuls -> Z [128=(qm,h), qG*128+w]
                pka = pka_tiles[wc]
                pkb = pkb_tiles[wc]
                z_sb = zsb.tile([128, NQG * 128], dt.bfloat16, tag="z_sb")
                for ch in range(4):
                    csl = slice(ch * 512, (ch + 1) * 512)
                    zp = spsum.tile([128, 512], dt.float32, tag="mm")
                    nc.tensor.matmul(zp[:], wsb["wblk_a"], pka[:, csl], start=True, stop=False)
                    nc.tensor.matmul(zp[:], wsb["wblk_b"], pkb[:, csl], start=False, stop=True)
                    if ch % 2 == 0:
                        nc.scalar.copy(z_sb[:, csl], zp[:])
                    else:
                        nc.vector.tensor_copy(z_sb[:, csl], zp[:])

                # transpose 4-chunk regions, then one strided scale per region:
                # bacc[h*768 + wc*256 + qg*16 + r] = zT[(r,h)] * rs[q]
                rs_bb = rs_b[:]
                bab = bacc_t[:]
                for rg in range(4):
                    rgn = trgn.tile([128, 512], dt.bfloat16, tag="rgn")
                    for k in range(4):
                        qg = rg * 4 + k
                        nc.tensor.transpose(
                            rgn[:, k * 128 : (k + 1) * 128],
                            z_sb[:, qg * 128 : (qg + 1) * 128], ident[:],
                        )
                    rgb = rgn[:]
                    zread = bass.AP(
                        tensor=rgb.tensor, offset=rgb.offset,
                        ap=[rgb.ap[0], [1, H], [H, 64]],
                    )
                    rsrd = bass.AP(
                        tensor=rs_bb.tensor, offset=rs_bb.offset + rg * 64,
                        ap=[rs_bb.ap[0], [0, H], [1, 64]],
                    )
                    dst = bass.AP(
                        tensor=bab.tensor,
                        offset=bab.offset + wc * NQ + rg * 64,
                        ap=[bab.ap[0], [NWC * NQ, H], [1, 64]],
                    )
                    nc.vector.tensor_tensor(dst, zread, rsrd, op=ALU.mult)

            # fold window/mask additive term into bacc (broadcast over heads)
            wab = wa_cat[:]
            bab = bacc_t[:]
            for wc in range(NWC):
                wsrc = bass.AP(
                    tensor=wab.tensor, offset=wab.offset + wc * NQ,
                    ap=[wab.ap[0], [0, H], [1, NQ]],
                )
                wdst = bass.AP(
                    tensor=bab.tensor, offset=bab.offset + wc * NQ,
                    ap=[bab.ap[0], [NWC * NQ, H], [1, NQ]],
                )
                nc.vector.tensor_tensor(wdst, wdst, wsrc, op=ALU.add)

            # ============ attention ============
            t2g = []
            for grp in range(2):
                t = sing.tile([128, NQ], dt.bfloat16, tag=f"t2g{grp}")
                nc.vector.memset(t[:], 0.0)
                t2g.append(t)
            for h in range(H):
                grp, hb = h // 4, (h % 4) * 32
                h32 = slice(hb, hb + 32)
                s_big = sbp.tile([128, NWC * NQ], dt.float32, tag="sbig")
                for wc in range(NWC):
                    wsl = slice(wc * 128, (wc + 1) * 128)
                    qslw = slice(wc * NQ, (wc + 1) * NQ)
                    nc.tensor.matmul(
                        s_big[:, qslw], ident[:],
                        bacc_t[:, h * NWC * NQ + wc * NQ : h * NWC * NQ + (wc + 1) * NQ],
                        start=True, stop=False,
                    )
                    nc.tensor.matmul(
                        s_big[:, qslw], k_Ts[grp][h32, wsl], q_Ts[grp][h32, :],
                        start=False, stop=True, tile_position=(hb, 0),
                    )
                p_big = stmp.tile([128, NWC * NQ], dt.bfloat16, tag="p_big")
                nc.scalar.activation(
                    p_big[:], s_big[:], AF.Exp, bias=c0_sb[:, h : h + 1]
                )
                ov = spsum.tile([VE, NQ], dt.float32, tag="mm")
                for wc in range(NWC):
                    nc.tensor.matmul(
                        ov[:], v_ext[wc][:, h * VE : (h + 1) * VE],
                        p_big[:, wc * NQ : (wc + 1) * NQ],
                        start=(wc == 0), stop=(wc == NWC - 1),
                    )
                zb_f = stmp.tile([D, NQ], dt.float32, tag="zb_f")
                nc.vector.reciprocal_approx_fast(zb_f[:], ov[0:D, :])
                u = stmp.tile([128, NQ], dt.bfloat16, tag="u")
                nc.vector.tensor_tensor(
                    u[hb : hb + D, :], zb_f[:], ov[32 : 32 + D, :], op=ALU.mult
                )
                nc.vector.tensor_tensor(
                    t2g[h // 4][hb : hb + D, :], u[hb : hb + D, :],
                    gq_Ts[h // 4][hb : hb + D, :], op=ALU.mult,
                )

            ao_ps = spsum.tile([128, NQ], dt.float32, tag="mm")
            nc.tensor.matmul(ao_ps[:], wsb["wo_a"], t2g[0][:], start=True, stop=False)
            nc.tensor.matmul(ao_ps[:], wsb["wo_b"], t2g[1][:], start=False, stop=True)
            attn = stmp.tile([128, NQ], dt.float32, tag="attn")
            nc.vector.scalar_tensor_tensor(
                attn[:], ao_ps[:], vsb["bo"], og_sb[:], ALU.add, ALU.mult
            )
            fin = stmp.tile([128, NQ], dt.float32, tag="fin")
            nc.vector.tensor_tensor(fin[:], attn[:], trans_g[:], op=ALU.add)

            # transpose [C, q] -> [q, C] and write out
            for qt in range(2):
                qts = slice(qt * 128, (qt + 1) * 128)
                otp = trgn.tile([128, 128], dt.float32, tag="rgn")
                nc.tensor.transpose(otp[:], fin[:, qts], ident_f[:])
                osb = stmp.tile([128, 128], dt.float32, tag="osb")
                nc.scalar.copy(osb[:], otp[:])
                nc.sync.dma_start(out=out_ext[qts, :], in_=osb[:])

    nc.finalize()
    return nc


def _prep(inputs):
    """Host-side shard + pack. Returns (in_maps, consts)."""
    f = {k: np.asarray(v) for k, v in inputs.items()}
    pair = f["atom_pair_repr"][0]          # [N, N, CP]
    asr = f["atom_single_repr"][0]         # [N, C]
    sp = f["atom_single_proj"][0]
    mask = f["mask"][0]                    # [N]

    ln_scale = f["pair_ln_scale"]
    ln_bias = f["pair_ln_bias"]
    wp_s = ln_scale[:, None] * f["w_pair"]           # [CP, H] scaled
    c0 = (ln_bias @ f["w_pair"]).astype(np.float64)  # [H]
    wp = wp_s - wp_s.sum(0, keepdims=True) / CP      # fold mean subtraction

    s_scale = f["adaln_s_scale"]
    t_scale = f["t_s_scale"]

    def bf(x):
        return np.ascontiguousarray(x.astype(BF16))

    def pad_heads(w, scale=1.0):
        a = np.zeros((C, 128), F32)
        b = np.zeros((C, 128), F32)
        for h4 in range(4):
            a[:, h4 * 32 : h4 * 32 + D] = w[:, h4 * D : (h4 + 1) * D] * scale
            b[:, h4 * 32 : h4 * 32 + D] = w[:, (h4 + 4) * D : (h4 + 5) * D] * scale
        return bf(a), bf(b)

    wq_a, wq_b = pad_heads(f["wq"], float(D) ** -0.5)
    wk_a, wk_b = pad_heads(f["wk"])
    wg_a, wg_b = pad_heads(f["wg"])

    def pad_rows(w):
        a = np.zeros((128, C), F32)
        b = np.zeros((128, C), F32)
        for h4 in range(4):
            a[h4 * 32 : h4 * 32 + D, :] = w[h4 * D : (h4 + 1) * D, :]
            b[h4 * 32 : h4 * 32 + D, :] = w[(h4 + 4) * D : (h4 + 5) * D, :]
        return bf(a), bf(b)

    wo_a, wo_b = pad_rows(f["wo"])
    bg_pad = np.zeros((2, 128), F32)
    for h4 in range(4):
        bg_pad[0, h4 * 32 : h4 * 32 + D] = f["bg"][h4 * D : (h4 + 1) * D]
        bg_pad[1, h4 * 32 : h4 * 32 + D] = f["bg"][(h4 + 4) * D : (h4 + 5) * D]

    wblk_a = np.zeros((128, 128), F32)
    wblk_b = np.zeros((128, 128), F32)
    for qm in range(8):
        wblk_a[qm * 16 : qm * 16 + 16, qm * 8 : qm * 8 + 8] = wp
        wblk_b[qm * 16 : qm * 16 + 16, 64 + qm * 8 : 64 + qm * 8 + 8] = wp

    weights = {
        "wq_a": wq_a, "wq_b": wq_b, "wk_a": wk_a, "wk_b": wk_b,
        "wg_a": wg_a, "wg_b": wg_b, "wo_a": wo_a, "wo_b": wo_b,
        "w_gate": bf(s_scale[:, None] * f["adaln_gate_w"]),
        "w_skip": bf(s_scale[:, None] * f["adaln_skip_w"]),
        "wv": bf(f["wv"]),
        "w_og": bf(f["out_gate_w"]),
        "w_tgate": bf(t_scale[:, None] * f["t_gate_w"]),
        "w_tskip": bf(t_scale[:, None] * f["t_skip_w"]),
        "w_sw0": bf(f["t_swish_w"][:, :128]), "w_sw1": bf(f["t_swish_w"][:, 128:]),
        "w_hd0": bf(f["t_hidden_w"][:, :128]), "w_hd1": bf(f["t_hidden_w"][:, 128:]),
        "w_to0": bf(f["t_out_w"][:128, :]), "w_to1": bf(f["t_out_w"][128:, :]),
        "w_tog": bf(f["t_out_gate_w"]),
        "wblk_a": bf(wblk_a), "wblk_b": bf(wblk_b),
    }
    wcat = np.concatenate([weights[k] for k in WNAMES], axis=1)
    vecs = {
        "gate_b": f["adaln_gate_b"], "bg_a": bg_pad[0], "bg_b": bg_pad[1],
        "bo": f["bo"],
        "og_b": f["out_gate_b"], "t_gate_b": f["t_gate_b"],
        "tog_b": f["t_out_gate_b"],
    }
    vcat = np.stack(
        [vecs[k].astype(F32).reshape(128) for k in VNAMES], axis=1
    )

    shared = {
        "wcat": np.ascontiguousarray(wcat),
        "vcat": np.ascontiguousarray(vcat),
    }

    in_maps = []
    qidx = np.arange(NQ)
    for i in range(NCORES):
        i0 = i * NQ
        lo = i0 - WH
        ks, ke = max(lo, 0), min(i0 + NQ + WH, N)
        strip = np.zeros((NQ, NW, CP), F32)
        strip[:, ks - lo : ke - lo] = pair[i0 : i0 + NQ, ks:ke]
        halo = np.zeros((NW, C), F32)
        halo_s = np.zeros((NW, C), F32)
        halo[ks - lo : ke - lo] = asr[ks:ke]
        halo_s[ks - lo : ke - lo] = sp[ks:ke]
        scat = np.concatenate(
            [x for rt in range(NWC)
             for x in (halo_s[rt * 128 : (rt + 1) * 128],
                       halo[rt * 128 : (rt + 1) * 128])],
            axis=1,
        )

        pw = strip.transpose(1, 0, 2).reshape(NW, NQ * CP)
        pair_w_i = bf(pw.reshape(NWC, 128, NQ * CP))

        s4 = strip.reshape(NQG, 16, NW, CP)          # [qG, qm, w, cp]
        def pack(half):
            t = s4[:, half * 8 : half * 8 + 8]       # [qG, 8, w, cp]
            t = t.transpose(1, 3, 0, 2)              # [qm, cp, qG, w]
            t = t.reshape(128, NQG, NWC, 128)        # [p, qG, wc, wl]
            return bf(t.transpose(2, 0, 1, 3).reshape(NWC, 128, NQG * 128))
        pk_a_i = pack(0)
        pk_b_i = pack(1)

        kabs = lo + np.arange(NW)
        inb = (kabs >= 0) & (kabs < N)
        mstrip = np.where(inb, mask[np.clip(kabs, 0, N - 1)], 0.0)
        valid = (
            (np.abs(kabs[:, None] - (i0 + qidx)[None, :]) <= WH)
            & inb[:, None]
            & (mstrip[:, None] > 0.5)
        )
        wa = np.where(valid, 0.0, NEG).reshape(NWC, 128, NQ)
        winadd_i = bf(wa.transpose(1, 0, 2).reshape(128, NWC * NQ))

        in_maps.append({
            **shared,
            "scat": np.ascontiguousarray(scat),
            "pair_w": pair_w_i,
            "pk_a": pk_a_i,
            "pk_b": pk_b_i,
            "winadd": winadd_i,
        })
    return in_maps, {"c0": c0}


_CACHE = {}


def kernel(**inputs):
    in_maps, consts = _prep(inputs)
    key = "graph"
    if key not in _CACHE:
        _CACHE[key] = build_graph(consts)
    nc = _CACHE[key]
    res = run_bass_kernel_spmd(nc, in_maps, core_ids=list(range(NCORES)))
    out = np.concatenate([res.results[i]["out"] for i in range(NCORES)], axis=0)
    return out.reshape(1, N, C).astype(np.float32)


if __name__ == "__main__":
    import reference

    ins = reference.setup_inputs()
    ins = {k: np.asarray(v) for k, v in ins.items()}
    got = kernel(**ins)
    exp = np.asarray(reference.reference(**reference.setup_inputs()))
    err = np.abs(got - exp).max() / (np.abs(exp).max() + 1e-9)
    print("Relative error:", err)


# revision 14
# speedup vs baseline: 1.3898x; 1.1351x over previous
"""Trainium2 Bass kernel for AtomTransformerBlock (sliding-window attention, W=64).

Sharding: 8 cores x 256 contiguous query atoms (sequence parallel), each core
gets a 64-atom halo of the singles and the corresponding pair-bias band strip.
No collectives needed (halo recompute). All heavy compute in bf16.

v2: packed weight/vec/singles DMAs, full pair prefetch, mean-subtraction folded
into the pair projection weights on host (bias = rs*(pair@wp') + c0), batched
bias-transpose scaling, winadd folded into the bias tile.
"""

import sys

sys.path.insert(0, "/opt/trn_rl_repo")

import numpy as np
import ml_dtypes

import concourse.bass as bass
import concourse.tile as tile
from concourse import bacc, masks, mybir
from concourse.bass_utils import run_bass_kernel_spmd

BF16 = ml_dtypes.bfloat16
F32 = np.float32

N = 2048
C = 128
CP = 16
H = 8
D = 16
WH = 64
NCORES = 8
NQ = N // NCORES          # 256 queries per core
NW = NQ + 2 * WH          # 384 window positions per core
NWC = NW // 128           # 3 w-chunks
NQG = NQ // 16            # 16 query groups of 16
EPS = 1e-5
NEG = -30000.0

dt = mybir.dt
AF = mybir.ActivationFunctionType
ALU = mybir.AluOpType
AX = mybir.AxisListType

WNAMES = [
    "w_gate", "w_skip", "wq_a", "wq_b", "wk_a", "wk_b", "wv",
    "wg_a", "wg_b", "wo_a", "wo_b", "w_og",
    "w_tgate", "w_tskip", "w_sw0", "w_sw1", "w_hd0", "w_hd1",
    "w_to0", "w_to1", "w_tog", "wblk_a", "wblk_b",
]
VNAMES = ["gate_b", "bg_a", "bg_b", "bo", "og_b", "t_gate_b", "tog_b"]


def build_graph(consts, skip=()):
    nc = bacc.Bacc(
        "TRN2",
        target_bir_lowering=False,
        debug=False,
        enable_asserts=False,
        num_devices=NCORES,
    )

    def inp(name, shape, dtype):
        return nc.dram_tensor(name, shape, dtype, kind="ExternalInput")

    # Per-core data (packed)
    scat_d = inp("scat", [128, 6 * 128], dt.float32)       # (sp,asr) x 3 row-tiles
    pair_w = inp("pair_w", [NWC, 128, NQ * CP], dt.bfloat16)
    pk_a = inp("pk_a", [NWC, 128, NQG * 128], dt.bfloat16)
    pk_b = inp("pk_b", [NWC, 128, NQG * 128], dt.bfloat16)
    winadd = inp("winadd", [128, NWC * NQ], dt.bfloat16)
    wcat_d = inp("wcat", [128, len(WNAMES) * 128], dt.bfloat16)
    vcat_d = inp("vcat", [128, len(VNAMES)], dt.float32)

    out_ext = nc.dram_tensor("out", [NQ, C], dt.float32, kind="ExternalOutput")

    c0 = consts["c0"]  # [H] python floats

    with tile.TileContext(nc) as tc:
        with (
            nc.allow_low_precision(reason="bf16 compute; tolerance 2e-2"),
            tc.tile_pool(name="const", bufs=1) as constp,
            tc.tile_pool(name="wpool", bufs=1) as wpool,
            tc.tile_pool(name="sing", bufs=1) as sing,
            tc.tile_pool(name="sbig", bufs=1) as sbig,
            tc.tile_pool(name="stmp", bufs=3) as stmp,
            tc.tile_pool(name="pairp", bufs=3) as pairp,
            tc.tile_pool(name="sqp", bufs=3) as sqp,
            tc.tile_pool(name="zsb", bufs=3) as zsb,
            tc.tile_pool(name="trgn", bufs=2, space="PSUM") as trgn,
            tc.tile_pool(name="spsum", bufs=2, space="PSUM") as spsum,
            tc.tile_pool(name="sbp", bufs=2, space="PSUM") as sbp,
        ):
            # --- packed input loads (few, large DMAs) ---
            scat = wpool.tile([128, 6 * 128], dt.float32, tag="scat")
            nc.sync.dma_start(out=scat[:], in_=scat_d[:])
            wcat = wpool.tile([128, len(WNAMES) * 128], dt.bfloat16, tag="wcat")
            nc.sync.dma_start(out=wcat[:], in_=wcat_d[:])
            vcat = wpool.tile([128, len(VNAMES)], dt.float32, tag="vcat")
            nc.sync.dma_start(out=vcat[:], in_=vcat_d[:])
            wa_cat = wpool.tile([128, NWC * NQ], dt.bfloat16, tag="wa_cat")
            nc.sync.dma_start(out=wa_cat[:], in_=winadd[:])

            # prefetch ALL pair data immediately (dedicated bufs => no gating)
            pw_tiles, pka_tiles, pkb_tiles = [], [], []
            for wc in range(NWC):
                pwt = pairp.tile([128, NQ * CP], dt.bfloat16, tag="pair_w")
                nc.sync.dma_start(out=pwt[:], in_=pair_w[wc])
                pw_tiles.append(pwt)
            for wc in range(NWC):
                pka = pairp.tile([128, NQG * 128], dt.bfloat16, tag="pka")
                nc.sync.dma_start(out=pka[:], in_=pk_a[wc])
                pka_tiles.append(pka)
                pkb = pairp.tile([128, NQG * 128], dt.bfloat16, tag="pkb")
                nc.sync.dma_start(out=pkb[:], in_=pk_b[wc])
                pkb_tiles.append(pkb)

            # --- constants ---
            zero_c = constp.tile([128, 1], dt.float32, tag="zero_c")
            nc.vector.memset(zero_c[:], 0.0)
            eps_c = constp.tile([128, 1], dt.float32, tag="eps_c")
            nc.vector.memset(eps_c[:], EPS)
            nc.const_aps.aps[(dt.float32, 0.0)] = zero_c[:]
            nc.const_aps.aps[(dt.float32, EPS)] = eps_c[:]
            ident = constp.tile([128, 128], dt.bfloat16)
            masks.make_identity(nc, ident[:])
            ident_f = constp.tile([128, 128], dt.float32, tag="ident_f")
            masks.make_identity(nc, ident_f[:])

            wsb = {k: wcat[:, i * 128 : (i + 1) * 128] for i, k in enumerate(WNAMES)}
            vsb = {k: vcat[:, i : i + 1] for i, k in enumerate(VNAMES)}
            c0_sb = wpool.tile([128, H], dt.float32, tag="c0")
            for h in range(H):
                nc.gpsimd.memset(c0_sb[:, h : h + 1], float(c0[h]))

            # ============ singles path ============
            # rows-on-partition LN, then PE-transpose into [C, rows] chain layout
            sn_T = sing.tile([128, NW], dt.bfloat16, tag="sn_T")
            lna_T = sing.tile([128, NW], dt.bfloat16, tag="lna_T")
            sp_T = sing.tile([128, NW], dt.bfloat16, tag="sp_T")
            for rt in range(NWC):
                rows = slice(rt * 128, (rt + 1) * 128)
                for ti, (nm, dstT, do_raw) in enumerate(
                    (("sp", sn_T, True), ("asr", lna_T, False))
                ):
                    xt = scat[:, (rt * 2 + ti) * 128 : (rt * 2 + ti + 1) * 128]
                    stats = stmp.tile([128, 6], dt.float32, tag="ln_stats")
                    nc.vector.bn_stats(out=stats[:], in_=xt)
                    mv = stmp.tile([128, 2], dt.float32, tag="ln_mv")
                    nc.vector.bn_aggr(out=mv[:], in_=stats[:])
                    sd = stmp.tile([128, 1], dt.float32, tag="ln_sd")
                    nc.scalar.activation(sd[:], mv[:, 1:2], AF.Sqrt, bias=EPS)
                    rsv = stmp.tile([128, 1], dt.float32, tag="ln_rs")
                    nc.vector.reciprocal_approx_fast(rsv[:], sd[:])
                    nmrs = stmp.tile([128, 1], dt.float32, tag="ln_nm")
                    nc.vector.scalar_tensor_tensor(
                        nmrs[:], mv[:, 0:1], -1.0, rsv[:], ALU.mult, ALU.mult
                    )
                    lnx = stmp.tile([128, 128], dt.bfloat16, tag="ln_out")
                    nc.scalar.activation(
                        lnx[:], xt, AF.Identity, bias=nmrs[:], scale=rsv[:]
                    )
                    tp = trgn.tile([128, 128], dt.bfloat16, tag="rgn")
                    nc.tensor.transpose(tp[:], lnx[:], ident[:])
                    nc.scalar.copy(dstT[:, rows], tp[:])
                    if do_raw:
                        tpr = trgn.tile([128, 128], dt.bfloat16, tag="rgn")
                        xb = stmp.tile([128, 128], dt.bfloat16, tag="ln_xb")
                        nc.scalar.copy(xb[:], xt)
                        nc.tensor.transpose(tpr[:], xb[:], ident[:])
                        nc.scalar.copy(sp_T[:, rows], tpr[:])

            # squares of pair (scalar engine for wc0/2, gpsimd wc1: keep DVE free)
            sq_tiles = []
            for wc in range(NWC):
                sq = sqp.tile([128, NQ * CP], dt.bfloat16, tag="sq")
                if wc == 1:
                    nc.gpsimd.tensor_tensor(
                        sq[:], pw_tiles[wc][:], pw_tiles[wc][:], op=ALU.mult
                    )
                else:
                    nc.scalar.square(sq[:], pw_tiles[wc][:])
                sq_tiles.append(sq)
            qsl = slice(WH, WH + NQ)  # center 256 of the 384 halo rows

            def mm(wname, rhs_ap, n, tag):
                ps = spsum.tile([128, n], dt.float32, tag=tag)
                nc.tensor.matmul(ps[:], wsb[wname], rhs_ap, start=True, stop=True)
                return ps

            # AdaLN for attention branch
            g_ps = mm("w_gate", sn_T[:], NW, "mm")
            gate_sb = stmp.tile([128, NW], dt.bfloat16, tag="gate")
            nc.scalar.activation(gate_sb[:], g_ps[:], AF.Sigmoid, bias=vsb["gate_b"])
            sk_ps = mm("w_skip", sn_T[:], NW, "mm")
            tg1 = stmp.tile([128, NW], dt.bfloat16, tag="atg1")
            nc.vector.tensor_tensor(tg1[:], gate_sb[:], lna_T[:], op=ALU.mult)
            a_T = sing.tile([128, NW], dt.bfloat16, tag="a_T")
            nc.vector.tensor_tensor(a_T[:], tg1[:], sk_ps[:], op=ALU.add)

            q_Ts, k_Ts = [], []
            for grp in ("a", "b"):
                q_ps = mm(f"wq_{grp}", a_T[:, qsl], NQ, "mm")
                q_Tg = sing.tile([128, NQ], dt.bfloat16, tag=f"q_T{grp}")
                nc.scalar.copy(q_Tg[:], q_ps[:])  # D^-0.5 folded into wq
                q_Ts.append(q_Tg)
                k_ps = mm(f"wk_{grp}", a_T[:], NW, "mm")
                k_Tg = sing.tile([128, NW], dt.bfloat16, tag=f"k_T{grp}")
                nc.scalar.copy(k_Tg[:], k_ps[:])
                k_Ts.append(k_Tg)
            v_ps = mm("wv", a_T[:], NW, "mm")
            v_T = sing.tile([128, NW], dt.bfloat16, tag="v_T")
            nc.scalar.copy(v_T[:], v_ps[:])
            gq_Ts = []
            for grp in ("a", "b"):
                g2_ps = mm(f"wg_{grp}", a_T[:, qsl], NQ, "mm")
                gq_Tg = sing.tile([128, NQ], dt.bfloat16, tag=f"gq_T{grp}")
                nc.scalar.activation(
                    gq_Tg[:], g2_ps[:], AF.Sigmoid, bias=vsb[f"bg_{grp}"]
                )
                gq_Ts.append(gq_Tg)

            # V_ext per w-chunk: per head 48 cols: [ones | 31 zero | V_h(16)]
            VE = 48
            v_ext = []
            for wc in range(NWC):
                tp = trgn.tile([128, 128], dt.bfloat16, tag="rgn")
                nc.tensor.transpose(tp[:], v_T[:, wc * 128 : (wc + 1) * 128], ident[:])
                ve = sing.tile([128, H * VE], dt.bfloat16, tag=f"vext{wc}")  # 1-buf pool: unique tags
                nc.gpsimd.memset(ve[:], 0.0)
                veb, tpb = ve[:], tp[:]
                ones_ap = bass.AP(
                    tensor=veb.tensor, offset=veb.offset,
                    ap=[veb.ap[0], [VE, H], [1, D]],
                )
                nc.gpsimd.memset(ones_ap, 1.0)
                dst = bass.AP(
                    tensor=veb.tensor, offset=veb.offset + 32,
                    ap=[veb.ap[0], [VE, H], [1, D]],
                )
                src = bass.AP(
                    tensor=tpb.tensor, offset=tpb.offset,
                    ap=[tpb.ap[0], [D, H], [1, D]],
                )
                nc.vector.tensor_copy(dst, src)
                v_ext.append(ve)

            # out gates (raw sp projections)
            og_ps = mm("w_og", sp_T[:, qsl], NQ, "mm")
            og_sb = stmp.tile([128, NQ], dt.bfloat16, tag="og")
            nc.scalar.activation(og_sb[:], og_ps[:], AF.Sigmoid, bias=vsb["og_b"])
            tog_ps = mm("w_tog", sp_T[:, qsl], NQ, "mm")
            tog_sb = stmp.tile([128, NQ], dt.bfloat16, tag="tog")
            nc.scalar.activation(tog_sb[:], tog_ps[:], AF.Sigmoid, bias=vsb["tog_b"])

            # transition branch
            tgp = mm("w_tgate", sn_T[:, qsl], NQ, "mm")
            tgate = stmp.tile([128, NQ], dt.bfloat16, tag="tgate")
            nc.scalar.activation(tgate[:], tgp[:], AF.Sigmoid, bias=vsb["t_gate_b"])
            tskp = mm("w_tskip", sn_T[:, qsl], NQ, "mm")
            tt1 = stmp.tile([128, NQ], dt.bfloat16, tag="tt1")
            nc.vector.tensor_tensor(tt1[:], tgate[:], lna_T[:, qsl], op=ALU.mult)
            t_T = sing.tile([128, NQ], dt.bfloat16, tag="t_T")
            nc.vector.tensor_tensor(t_T[:], tt1[:], tskp[:], op=ALU.add)

            hid = []
            for half in range(2):
                swp = mm(f"w_sw{half}", t_T[:], NQ, "mm")
                sw = stmp.tile([128, NQ], dt.bfloat16, tag=f"sw{half}")
                nc.scalar.activation(sw[:], swp[:], AF.Silu)
                hdp = mm(f"w_hd{half}", t_T[:], NQ, "mm")
                hh = sing.tile([128, NQ], dt.bfloat16, tag=f"hid{half}")
                nc.vector.tensor_tensor(hh[:], sw[:], hdp[:], op=ALU.mult)
                hid.append(hh)
            tr_ps = spsum.tile([128, NQ], dt.float32, tag="mm")
            nc.tensor.matmul(tr_ps[:], wsb["w_to0"], hid[0][:], start=True, stop=False)
            nc.tensor.matmul(tr_ps[:], wsb["w_to1"], hid[1][:], start=False, stop=True)
            trans_g = sing.tile([128, NQ], dt.float32, tag="trans_g")
            nc.vector.tensor_tensor(trans_g[:], tog_sb[:], tr_ps[:], op=ALU.mult)

            # ============ pair path ============
            # LN stats: mean and rstd per (w, q). Mean-subtraction of the bias
            # itself is folded into wblk on host; m only feeds the variance.
            bacc_t = sbig.tile([128, H * NWC * NQ], dt.bfloat16, tag="bacc")
            rs_bs = []
            for wc in range(NWC):
                pwt = pw_tiles[wc]
                pw3 = pwt[:].rearrange("p (q c) -> p q c", c=CP)
                sums = stmp.tile([128, NQ], dt.bfloat16, tag="sums")
                nc.vector.tensor_reduce(out=sums[:], in_=pw3, axis=AX.X, op=ALU.add)
                sq = sq_tiles[wc]
                sumsq = stmp.tile([128, NQ], dt.bfloat16, tag="sumsq")
                nc.vector.tensor_reduce(
                    out=sumsq[:], in_=sq[:].rearrange("p (q c) -> p q c", c=CP),
                    axis=AX.X, op=ALU.add,
                )
                m = stmp.tile([128, NQ], dt.float32, tag="m")
                nc.vector.tensor_scalar_mul(m[:], sums[:], 1.0 / CP)
                nm2 = stmp.tile([128, NQ], dt.float32, tag="nm2")
                nc.vector.scalar_tensor_tensor(
                    nm2[:], m[:], -1.0, m[:], ALU.mult, ALU.mult
                )
                var = stmp.tile([128, NQ], dt.float32, tag="var")
                nc.vector.scalar_tensor_tensor(
                    var[:], sumsq[:], 1.0 / CP, nm2[:], ALU.mult, ALU.add
                )
                varc = stmp.tile([128, NQ], dt.float32, tag="varc")
                nc.vector.tensor_scalar_max(varc[:], var[:], 0.0)
                sd = stmp.tile([128, NQ], dt.float32, tag="sdp")
                nc.scalar.activation(sd[:], varc[:], AF.Sqrt, bias=EPS)
                rs = stmp.tile([128, NQ], dt.float32, tag="rsp")
                nc.vector.reciprocal_approx_fast(rs[:], sd[:])
                rs_b = stmp.tile([128, NQ], dt.bfloat16, tag="rsb")
                nc.scalar.copy(rs_b[:], rs[:])
                rs_bs.append(rs_b)

                # packed z matmuls -> Z [128=(qm,h), qG*128+w]
                pka = pka_tiles[wc]
                pkb = pkb_tiles[wc]
                z_sb = zsb.tile([128, NQG * 128], dt.bfloat16, tag="z_sb")
                for ch in range(4):
                    csl = slice(ch * 512, (ch + 1) * 512)
                    zp = spsum.tile([128, 512], dt.float32, tag="mm")
                    nc.tensor.matmul(zp[:], wsb["wblk_a"], pka[:, csl], start=True, stop=False)
                    nc.tensor.matmul(zp[:], wsb["wblk_b"], pkb[:, csl], start=False, stop=True)
                    nc.scalar.copy(z_sb[:, csl], zp[:])

                # transpose 4-chunk regions, then one strided scale per region:
                # bacc[h*768 + wc*256 + qg*16 + r] = zT[(r,h)] * rs[q]
                rs_bb = rs_b[:]
                bab = bacc_t[:]
                for rg in range(4):
                    rgn = trgn.tile([128, 512], dt.bfloat16, tag="rgn")
                    for k in range(4):
                        qg = rg * 4 + k
                        nc.tensor.transpose(
                            rgn[:, k * 128 : (k + 1) * 128],
                            z_sb[:, qg * 128 : (qg + 1) * 128], ident[:],
                        )
                    rgb = rgn[:]
                    zread = bass.AP(
                        tensor=rgb.tensor, offset=rgb.offset,
                        ap=[rgb.ap[0], [1, H], [H, 64]],
                    )
                    rsrd = bass.AP(
                        tensor=rs_bb.tensor, offset=rs_bb.offset + rg * 64,
                        ap=[rs_bb.ap[0], [0, H], [1, 64]],
                    )
                    dst = bass.AP(
                        tensor=bab.tensor,
                        offset=bab.offset + wc * NQ + rg * 64,
                        ap=[bab.ap[0], [NWC * NQ, H], [1, 64]],
                    )
                    nc.vector.tensor_tensor(dst, zread, rsrd, op=ALU.mult)

            # fold window/mask additive term into bacc (broadcast over heads)
            wab = wa_cat[:]
            bab = bacc_t[:]
            for wc in range(NWC):
                wsrc = bass.AP(
                    tensor=wab.tensor, offset=wab.offset + wc * NQ,
                    ap=[wab.ap[0], [0, H], [1, NQ]],
                )
                wdst = bass.AP(
                    tensor=bab.tensor, offset=bab.offset + wc * NQ,
                    ap=[bab.ap[0], [NWC * NQ, H], [1, NQ]],
                )
                nc.vector.tensor_tensor(wdst, wdst, wsrc, op=ALU.add)

            # ============ attention ============
            t2g = []
            for grp in range(2):
                t = sing.tile([128, NQ], dt.bfloat16, tag=f"t2g{grp}")
                nc.gpsimd.memset(t[:], 0.0)
                t2g.append(t)
            for h in range(H):
                grp, hb = h // 4, (h % 4) * 32
                h32 = slice(hb, hb + 32)
                s_big = sbp.tile([128, NWC * NQ], dt.float32, tag="sbig")
                for wc in range(NWC):
                    wsl = slice(wc * 128, (wc + 1) * 128)
                    qslw = slice(wc * NQ, (wc + 1) * NQ)
                    nc.tensor.matmul(
                        s_big[:, qslw], ident[:],
                        bacc_t[:, h * NWC * NQ + wc * NQ : h * NWC * NQ + (wc + 1) * NQ],
                        start=True, stop=False,
                    )
                    nc.tensor.matmul(
                        s_big[:, qslw], k_Ts[grp][h32, wsl], q_Ts[grp][h32, :],
                        start=False, stop=True, tile_position=(hb, 0),
                    )
                p_big = stmp.tile([128, NWC * NQ], dt.bfloat16, tag="p_big")
                nc.scalar.activation(
                    p_big[:], s_big[:], AF.Exp, bias=c0_sb[:, h : h + 1]
                )
                ov = spsum.tile([VE, NQ], dt.float32, tag="mm")
                for wc in range(NWC):
                    nc.tensor.matmul(
                        ov[:], v_ext[wc][:, h * VE : (h + 1) * VE],
                        p_big[:, wc * NQ : (wc + 1) * NQ],
                        start=(wc == 0), stop=(wc == NWC - 1),
                    )
                zb_f = stmp.tile([D, NQ], dt.float32, tag="zb_f")
                nc.vector.reciprocal_approx_fast(zb_f[:], ov[0:D, :])
                u = stmp.tile([128, NQ], dt.bfloat16, tag="u")
                nc.vector.tensor_tensor(
                    u[hb : hb + D, :], zb_f[:], ov[32 : 32 + D, :], op=ALU.mult
                )
                nc.vector.tensor_tensor(
                    t2g[h // 4][hb : hb + D, :], u[hb : hb + D, :],
                    gq_Ts[h // 4][hb : hb + D, :], op=ALU.mult,
                )

            ao_ps = spsum.tile([128, NQ], dt.float32, tag="mm")
            nc.tensor.matmul(ao_ps[:], wsb["wo_a"], t2g[0][:], start=True, stop=False)
            nc.tensor.matmul(ao_ps[:], wsb["wo_b"], t2g[1][:], start=False, stop=True)
            attn = stmp.tile([128, NQ], dt.float32, tag="attn")
            nc.vector.scalar_tensor_tensor(
                attn[:], ao_ps[:], vsb["bo"], og_sb[:], ALU.add, ALU.mult
            )
            fin = stmp.tile([128, NQ], dt.float32, tag="fin")
            nc.vector.tensor_tensor(fin[:], attn[:], trans_g[:], op=ALU.add)

            # transpose [C, q] -> [q, C] and write out
            for qt in range(2):
                qts = slice(qt * 128, (qt + 1) * 128)
                otp = trgn.tile([128, 128], dt.float32, tag="rgn")
                nc.tensor.transpose(otp[:], fin[:, qts], ident_f[:])
                osb = stmp.tile([128, 128], dt.float32, tag="osb")
                nc.scalar.copy(osb[:], otp[:])
                nc.sync.dma_start(out=out_ext[qts, :], in_=osb[:])

    nc.finalize()
    return nc


def _prep(inputs):
    """Host-side shard + pack. Returns (in_maps, consts)."""
    f = {k: np.asarray(v) for k, v in inputs.items()}
    pair = f["atom_pair_repr"][0]          # [N, N, CP]
    asr = f["atom_single_repr"][0]         # [N, C]
    sp = f["atom_single_proj"][0]
    mask = f["mask"][0]                    # [N]

    ln_scale = f["pair_ln_scale"]
    ln_bias = f["pair_ln_bias"]
    wp_s = ln_scale[:, None] * f["w_pair"]           # [CP, H] scaled
    c0 = (ln_bias @ f["w_pair"]).astype(np.float64)  # [H]
    wp = wp_s - wp_s.sum(0, keepdims=True) / CP      # fold mean subtraction

    s_scale = f["adaln_s_scale"]
    t_scale = f["t_s_scale"]

    def bf(x):
        return np.ascontiguousarray(x.astype(BF16))

    def pad_heads(w, scale=1.0):
        a = np.zeros((C, 128), F32)
        b = np.zeros((C, 128), F32)
        for h4 in range(4):
            a[:, h4 * 32 : h4 * 32 + D] = w[:, h4 * D : (h4 + 1) * D] * scale
            b[:, h4 * 32 : h4 * 32 + D] = w[:, (h4 + 4) * D : (h4 + 5) * D] * scale
        return bf(a), bf(b)

    wq_a, wq_b = pad_heads(f["wq"], float(D) ** -0.5)
    wk_a, wk_b = pad_heads(f["wk"])
    wg_a, wg_b = pad_heads(f["wg"])

    def pad_rows(w):
        a = np.zeros((128, C), F32)
        b = np.zeros((128, C), F32)
        for h4 in range(4):
            a[h4 * 32 : h4 * 32 + D, :] = w[h4 * D : (h4 + 1) * D, :]
            b[h4 * 32 : h4 * 32 + D, :] = w[(h4 + 4) * D : (h4 + 5) * D, :]
        return bf(a), bf(b)

    wo_a, wo_b = pad_rows(f["wo"])
    bg_pad = np.zeros((2, 128), F32)
    for h4 in range(4):
        bg_pad[0, h4 * 32 : h4 * 32 + D] = f["bg"][h4 * D : (h4 + 1) * D]
        bg_pad[1, h4 * 32 : h4 * 32 + D] = f["bg"][(h4 + 4) * D : (h4 + 5) * D]

    wblk_a = np.zeros((128, 128), F32)
    wblk_b = np.zeros((128, 128), F32)
    for qm in range(8):
        wblk_a[qm * 16 : qm * 16 + 16, qm * 8 : qm * 8 + 8] = wp
        wblk_b[qm * 16 : qm * 16 + 16, 64 + qm * 8 : 64 + qm * 8 + 8] = wp

    weights = {
        "wq_a": wq_a, "wq_b": wq_b, "wk_a": wk_a, "wk_b": wk_b,
        "wg_a": wg_a, "wg_b": wg_b, "wo_a": wo_a, "wo_b": wo_b,
        "w_gate": bf(s_scale[:, None] * f["adaln_gate_w"]),
        "w_skip": bf(s_scale[:, None] * f["adaln_skip_w"]),
        "wv": bf(f["wv"]),
        "w_og": bf(f["out_gate_w"]),
        "w_tgate": bf(t_scale[:, None] * f["t_gate_w"]),
        "w_tskip": bf(t_scale[:, None] * f["t_skip_w"]),
        "w_sw0": bf(f["t_swish_w"][:, :128]), "w_sw1": bf(f["t_swish_w"][:, 128:]),
        "w_hd0": bf(f["t_hidden_w"][:, :128]), "w_hd1": bf(f["t_hidden_w"][:, 128:]),
        "w_to0": bf(f["t_out_w"][:128, :]), "w_to1": bf(f["t_out_w"][128:, :]),
        "w_tog": bf(f["t_out_gate_w"]),
        "wblk_a": bf(wblk_a), "wblk_b": bf(wblk_b),
    }
    wcat = np.concatenate([weights[k] for k in WNAMES], axis=1)
    vecs = {
        "gate_b": f["adaln_gate_b"], "bg_a": bg_pad[0], "bg_b": bg_pad[1],
        "bo": f["bo"],
        "og_b": f["out_gate_b"], "t_gate_b": f["t_gate_b"],
        "tog_b": f["t_out_gate_b"],
    }
    vcat = np.stack(
        [vecs[k].astype(F32).reshape(128) for k in VNAMES], axis=1
    )

    shared = {
        "wcat": np.ascontiguousarray(wcat),
        "vcat": np.ascontiguousarray(vcat),
    }

    in_maps = []
    qidx = np.arange(NQ)
    for i in range(NCORES):
        i0 = i * NQ
        lo = i0 - WH
        ks, ke = max(lo, 0), min(i0 + NQ + WH, N)
        strip = np.zeros((NQ, NW, CP), F32)
        strip[:, ks - lo : ke - lo] = pair[i0 : i0 + NQ, ks:ke]
        halo = np.zeros((NW, C), F32)
        halo_s = np.zeros((NW, C), F32)
        halo[ks - lo : ke - lo] = asr[ks:ke]
        halo_s[ks - lo : ke - lo] = sp[ks:ke]
        scat = np.concatenate(
            [x for rt in range(NWC)
             for x in (halo_s[rt * 128 : (rt + 1) * 128],
                       halo[rt * 128 : (rt + 1) * 128])],
            axis=1,
        )

        pw = strip.transpose(1, 0, 2).reshape(NW, NQ * CP)
        pair_w_i = bf(pw.reshape(NWC, 128, NQ * CP))

        s4 = strip.reshape(NQG, 16, NW, CP)          # [qG, qm, w, cp]
        def pack(half):
            t = s4[:, half * 8 : half * 8 + 8]       # [qG, 8, w, cp]
            t = t.transpose(1, 3, 0, 2)              # [qm, cp, qG, w]
            t = t.reshape(128, NQG, NWC, 128)        # [p, qG, wc, wl]
            return bf(t.transpose(2, 0, 1, 3).reshape(NWC, 128, NQG * 128))
        pk_a_i = pack(0)
        pk_b_i = pack(1)

        kabs = lo + np.arange(NW)
        inb = (kabs >= 0) & (kabs < N)
        mstrip = np.where(inb, mask[np.clip(kabs, 0, N - 1)], 0.0)
        valid = (
            (np.abs(kabs[:, None] - (i0 + qidx)[None, :]) <= WH)
            & inb[:, None]
            & (mstrip[:, None] > 0.5)
        )
        wa = np.where(valid, 0.0, NEG).reshape(NWC, 128, NQ)
        winadd_i = bf(wa.transpose(1, 0, 2).reshape(128, NWC * NQ))

        in_maps.append({
            **shared,
            "scat": np.ascontiguousarray(scat),
            "pair_w": pair_w_i,
            "pk_a": pk_a_i,
            "pk_b": pk_b_i,
            "winadd": winadd_i,
        })
    return in_maps, {"c0": c0}


_CACHE = {}


def kernel(**inputs):
    in_maps, consts = _prep(inputs)
    key = "graph"
    if key not in _CACHE:
        _CACHE[key] = build_graph(consts)
    nc = _CACHE[key]
    res = run_bass_kernel_spmd(nc, in_maps, core_ids=list(range(NCORES)))
    out = np.concatenate([res.results[i]["out"] for i in range(NCORES)], axis=0)
    return out.reshape(1, N, C).astype(np.float32)


if __name__ == "__main__":
    import reference

    ins = reference.setup_inputs()
    ins = {k: np.asarray(v) for k, v in ins.items()}
    got = kernel(**ins)
    exp = np.asarray(reference.reference(**reference.setup_inputs()))
    err = np.abs(got - exp).max() / (np.abs(exp).max() + 1e-9)
    print("Relative error:", err)
